# revision 33
# baseline (speedup 1.0000x reference)
"""Trainium2 Bass kernel for DeChunking EMA (lower-triangular decay matmul).

Math: out[b,i,:] = sum_{j<=i} exp(S_i - S_j) * p_j * z[b,j,:],
with S = cumsum(log(clip(1-p))). Computed chunked-scan style (Mamba-SSD):

  - L split into C=32 chunks of Q=128.
  - Intra-chunk: out_intra = W_c^T.T @ z_c with
      W_c^T[j,i] = exp(S'_i - S'_j + log p_j) (masked to i>=j),
    where S' is S re-centered per chunk (only within-chunk differences
    matter, and small magnitudes survive the PE's fp32r mantissa split).
    The delta matrix is produced on PE by a block-diagonal stacked fp32r
    matmul: delta = 1*S'_i + (-S'_j)*1 + logp_j*1, K=3 per chunk -> K=12
    block-diagonal over a group of 4 chunks ([128,512] PSUM, 1 cycle/row).
  - Inter-chunk: chunk states H_c = U_c^T @ z_c accumulate into one
    [32,192] PSUM tile via a block-diagonal U (zero except column c of
    each [128,32] slab), then one [32,32] decay matmul forms all carry-in
    rows (carry = M2^T @ H), applied per chunk as a rank-1 PSUM-accumulated
    matmul out += A_c (x) carry_c.

All exp inputs are <= 0 by construction, so nothing overflows. The decay
weights / z / state operands run in bf16 on the PE (fp32 PSUM accumulate);
the delta stack runs in fp32r (S' re-centered + pre-rounded to bf16 hi+lo).

DRAM layouts are position-major ([Q, C*DBLK]) so every DMA moves >=3 KiB
contiguous per partition.

Sharding (8 cores, no collectives): core = (batch b in {0,1}) x (one of 4
D-blocks of 192). Each core reads z[b, :, blk] and pt[b] only.
"""

import os
import numpy as np
import ml_dtypes

B, L, D = 2, 4096, 768
Q = 128
C = L // Q           # 32 chunks
ND = 4               # D blocks per batch
DBLK = D // ND       # 192
GRP = 4              # chunks per delta/exp group
NG = C // GRP        # 8 groups
NEG = -3.0e38
N_CORES = 8
NZDMA = 2            # z-load DMA splits (6 KiB bursts/partition)
NODMA = 4            # out-store DMA splits

_CTX = {}
LAST_EXEC_NS = None


def _build_program():
    import concourse.bacc as bacc
    import concourse.mybir as mybir
    from concourse import tile

    f32 = mybir.dt.float32
    f32r = mybir.dt.float32r
    bf16 = mybir.dt.bfloat16
    nc = bacc.Bacc("TRN2", target_bir_lowering=False, debug=False,
                   num_devices=N_CORES, num_swdge_queues=4)

    FD = C * DBLK  # 6144 free elems in the big position-major tiles
    SLAB = C // NZDMA          # 8 chunks per z slab tile
    ZSL = SLAB * DBLK          # free elems per slab
    z_s = nc.dram_tensor("z_s", [Q, FD], bf16, kind="ExternalInput")
    # packed aux inputs: one DMA per partition-count class
    aux12 = nc.dram_tensor("aux12", [3 * GRP, NG * Q + NG * GRP * Q], f32r,
                           kind="ExternalInput")
    aux128 = nc.dram_tensor("aux128", [Q, C], f32, kind="ExternalInput")
    auxw = nc.dram_tensor("auxw", [Q, GRP * Q + Q], bf16, kind="ExternalInput")
    aux32 = nc.dram_tensor("aux32", [C, C], f32, kind="ExternalInput")
    out_s = nc.dram_tensor("out_s", [Q, FD], bf16, kind="ExternalOutput")

    Exp = mybir.ActivationFunctionType.Exp

    with tile.TileContext(nc) as tc:
        with (
            tc.tile_pool(name="zp", bufs=NZDMA) as zp,
            tc.tile_pool(name="wp", bufs=NG) as wp,
            tc.tile_pool(name="sp", bufs=1) as sp,
            tc.tile_pool(name="dps", bufs=2, space="PSUM") as dps,
            tc.tile_pool(name="ops", bufs=3, space="PSUM") as ops,
            tc.tile_pool(name="hps", bufs=1, space="PSUM") as hps,
        ):
            # z slabs stream first on sync (they gate the H state matmuls);
            # aux12 follows (the delta matmuls need it only once H is done)
            zsl = []
            for s in range(NZDMA):
                t = zp.tile([Q, ZSL], bf16, tag="z")
                nc.sync.dma_start(t[:], z_s[:, s * ZSL : (s + 1) * ZSL])
                zsl.append(t)
            a12 = sp.tile([3 * GRP, NG * Q + NG * GRP * Q], f32r, tag="a12")
            nc.sync.dma_start(a12[:], aux12[:])

            def zchunk(c):
                s, r = divmod(c, SLAB)
                return zsl[s][:, r * DBLK : (r + 1) * DBLK]

            # remaining aux loads ride the Activation HWDGE queue in
            # parallel, smallest (most urgent) first
            a128 = sp.tile([Q, C], f32, tag="a128")
            nc.scalar.dma_start(a128[:], aux128[:])
            a32 = sp.tile([C, C], f32, tag="a32")
            nc.scalar.dma_start(a32[:], aux32[:])
            aw = sp.tile([Q, GRP * Q + Q], bf16, tag="aw")
            nc.scalar.dma_start(aw[:], auxw[:])
            sL = a12[:, 0 : NG * Q]
            sR = a12[:, NG * Q :]
            ue = a128[:]
            d2 = a32[:]
            utri = aw[:, 0 : GRP * Q]
            idn = aw[:, GRP * Q :]

            # U block-diagonal [Q, C*C] bf16: zero it, exp the [Q, C] column
            # stack, scatter onto the diagonal (stride C+1 in the free dim)
            Ublk = sp.tile([Q, C * C], bf16, tag="Ublk")
            nc.gpsimd.memset(Ublk[:], 0.0)
            Us = sp.tile([Q, C], bf16, tag="Us")
            nc.scalar.activation(Us[:], ue, Exp)
            nc.scalar.copy(Ublk[:, 0 : C * C : C + 1], Us[:])
            M2 = sp.tile([C, C], bf16, tag="M2")
            nc.scalar.activation(M2[:], d2, Exp)

            # PE clock warmup: back-to-back dense matmuls on junk data during
            # the input-DMA window flip the HAM gate to 2.4 GHz before the
            # real work arrives (the real matmuls alone are too sparse in
            # array-duty to flip it); the real H matmuls then sustain it
            wm_sb = sp.tile([Q, 2 * DBLK], bf16, tag="wm_sb")
            nc.gpsimd.memset(wm_sb[:], 1.0)
            wm_ps = ops.tile([Q, 2 * DBLK], f32, tag="o")
            for _ in range(14):
                nc.tensor.matmul(wm_ps[:], wm_sb[:, 0:Q], wm_sb[:])

            # H state matmuls chase the z slabs as they land
            h_ps = hps.tile([C, DBLK], f32, tag="h")
            for c in range(C):
                nc.tensor.matmul(
                    h_ps[:],
                    Ublk[:, c * C : (c + 1) * C],
                    zchunk(c),
                    start=(c == 0), stop=(c == C - 1),
                    skip_group_check=True,
                )

            H = sp.tile([C, DBLK], bf16, tag="H")
            nc.vector.tensor_copy(H[:], h_ps[:])
            c_ps = hps.tile([C, DBLK], f32, tag="cps")
            nc.tensor.matmul(c_ps[:], M2[:], H[:])
            # kappa is pre-folded into M2 host-side, so c_ps already holds
            # kappa*carry; cast it and fold into row 0 of each z slab
            # (out += a (x) carry == W^T row 0 applying the rank-1 update
            # once z[0] += kappa*carry)
            cfk = sp.tile([C, DBLK], bf16, tag="cfk")
            nc.vector.tensor_copy(cfk[:], c_ps[:])
            for s in range(NZDMA):
                nc.gpsimd.dma_start(
                    zsl[s][0:1, :],
                    cfk[s * SLAB : (s + 1) * SLAB, :],
                    accum_op=mybir.AluOpType.add,
                )

            # W^T delta matmuls (block-diagonal K=12 fp32r, N=512 -> 1
            # cycle/row) + mask + exp fill the PE while the fold completes;
            # intra-chunk output matmuls trail two delta groups behind
            wT = []
            osb = sp.tile([Q, FD], bf16, tag="osb")

            def delta_group(g):
                dp = dps.tile([Q, GRP * Q], f32, tag="dp")
                nc.tensor.matmul(
                    dp[:],
                    sL[:, g * Q : (g + 1) * Q],
                    sR[:, g * GRP * Q : (g + 1) * GRP * Q],
                    start=True, stop=False,
                )
                # tril mask applied on PE: accumulate identity @ utri_neg
                nc.tensor.matmul(dp[:], idn, utri, start=False, stop=True)
                w4 = wp.tile([Q, GRP * Q], bf16, tag="w4")
                nc.scalar.activation(w4[:], dp[:], Exp)
                wT.append(w4)

            def out_pair(p):
                o_ps = ops.tile([Q, 2 * DBLK], f32, tag="o")
                for h in range(2):
                    c = 2 * p + h
                    g, k = divmod(c, GRP)
                    nc.tensor.matmul(
                        o_ps[:, h * DBLK : (h + 1) * DBLK],
                        wT[g][:, k * Q : (k + 1) * Q],
                        zchunk(c),
                    )
                osl = slice(2 * p * DBLK, (2 * p + 2) * DBLK)
                if p % 5 == 4:
                    nc.scalar.copy(osb[:, osl], o_ps[:])
                else:
                    nc.vector.tensor_copy(osb[:, osl], o_ps[:])

            delta_group(0)
            delta_group(1)
            for g in range(2, NG):
                out_pair(2 * (g - 2))
                out_pair(2 * (g - 2) + 1)
                delta_group(g)
            for p in range(2 * (NG - 2), C // 2):
                out_pair(p)

            ssl = FD // NODMA
            for s in range(NODMA):
                nc.sync.dma_start(
                    out_s[:, s * ssl : (s + 1) * ssl],
                    osb[:, s * ssl : (s + 1) * ssl],
                )

    nc.compile()
    return nc


def _host_prep(pt_b):
    """Per-batch host-side prep of the small scan operands. pt_b: [L] f32."""
    pt_b = pt_b.astype(np.float64)
    decay = np.clip(1.0 - pt_b, 1e-12, None)
    S = np.cumsum(np.log(decay))
    logp = np.log(np.maximum(pt_b, 1e-38))
    Send = S[Q - 1 :: Q]
    Sendprev = np.concatenate([[0.0], Send[:-1]])

    Sm = S.reshape(C, Q)
    logpm = logp.reshape(C, Q)
    # Re-center S within each chunk (see module docstring) and pre-round
    # operands to bf16-hi+lo representable values so the fp32r matmul
    # decomposition is exact.
    Sc = Sm - Sm[:, :1]

    def r16(x):
        h = x.astype(ml_dtypes.bfloat16).astype(np.float64)
        l = (x - h).astype(ml_dtypes.bfloat16).astype(np.float64)
        return h + l

    Sc = r16(Sc)
    logpr = r16(logpm)

    stackL = np.zeros((3 * GRP, NG * Q), np.float32)
    stackR = np.zeros((3 * GRP, NG * GRP * Q), np.float32)
    for g in range(NG):
        for k in range(GRP):
            c = g * GRP + k
            lcol = slice(g * Q, (g + 1) * Q)
            stackL[3 * k + 0, lcol] = 1.0
            stackL[3 * k + 1, lcol] = -Sc[c]
            stackL[3 * k + 2, lcol] = logpr[c]
            rcol = slice(g * GRP * Q + k * Q, g * GRP * Q + (k + 1) * Q)
            stackR[3 * k + 0, rcol] = Sc[c]
            stackR[3 * k + 1, rcol] = 1.0
            stackR[3 * k + 2, rcol] = 1.0

    # U exponent column stack: Send_c - S_j + logp_j  -> [Q, C]
    uexp = (Send[:, None] - Sm + logpm).T.astype(np.float32)

    m_i = np.arange(C)[:, None]
    c_i = np.arange(C)[None, :]
    d2exp = np.where(m_i < c_i, Sendprev[None, :] - Send[:, None], NEG)
    d2exp = d2exp.astype(np.float32)

    # log kappa_c = S_{c,0} - Send_{c-1} - logp_r[c,0]: scaling such that
    # W^T row 0 (= exp(S'_i + logp_r[c,0])) times kappa*carry reproduces the
    # rank-1 carry term a_i*carry. Uses the device-rounded logp so the
    # coefficient reconstruction cancels exactly. Folded into the M2 decay
    # matrix exponents host-side (column c of d2exp).
    logkap = np.minimum(Sm[:, 0] - Sendprev - logpr[:, 0], 69.0)
    d2exp = (d2exp + logkap[None, :]).astype(np.float32)

    aux12 = np.concatenate([stackL, stackR], axis=1)
    aux128 = uexp
    aux32 = d2exp
    return aux12, aux128, aux32


_AUXW = None


def _get_auxw():
    """bf16 [Q, GRP*Q + Q]: tiled strict-upper NEG mask + identity."""
    global _AUXW
    if _AUXW is None:
        j = np.arange(Q)[:, None]
        i = np.arange(Q)[None, :]
        one = np.where(i >= j, 0.0, NEG)
        utri = np.tile(one, (1, GRP))
        arr = np.concatenate([utri, np.eye(Q)], axis=1)
        _AUXW = arr.astype(ml_dtypes.bfloat16)
    return _AUXW


def _make_in_maps(z, pt):
    preps = [_host_prep(pt[b]) for b in range(B)]
    in_maps = []
    for core in range(N_CORES):
        b, dblk = divmod(core, ND)
        aux12, aux128, aux32 = preps[b]
        z_slab = (
            z[b, :, dblk * DBLK : (dblk + 1) * DBLK]
            .reshape(C, Q, DBLK)
            .transpose(1, 0, 2)
            .reshape(Q, C * DBLK)
            .astype(ml_dtypes.bfloat16)
        )
        in_maps.append({
            "z_s": np.ascontiguousarray(z_slab),
            "aux12": aux12,
            "aux128": aux128,
            "aux32": aux32,
            "auxw": _get_auxw(),
        })
    return in_maps


def _unpack_out(res_core):
    """out_s [Q, C*DBLK] bf16 position-major -> [L, DBLK] f32."""
    return (
        res_core.astype(np.float32)
        .reshape(Q, C, DBLK)
        .transpose(1, 0, 2)
        .reshape(L, DBLK)
    )


def _install_ntff_shim():
    """Enable NTFF profiling under axon: shim the missing antenv.axon_hooks
    module and register the ctypes hook from trn_boot; skip the fileshare
    artifact upload (no bucket in this container)."""
    import sys
    import types
    import antenv

    if "antenv.axon_hooks" not in sys.modules:
        mod = types.ModuleType("antenv.axon_hooks")
        hook_box = [None]
        mod.set_axon_ntff_profile_hook = lambda h: hook_box.__setitem__(0, h)
        mod.get_axon_ntff_profile_hook = lambda: hook_box[0]
        mod._hook_box = hook_box
        sys.modules["antenv.axon_hooks"] = mod
        antenv.axon_hooks = mod
    mod = sys.modules["antenv.axon_hooks"]
    if mod.get_axon_ntff_profile_hook() is None:
        from trn_agent_boot.trn_boot import _ntff_profile_via_ctypes

        mod.set_axon_ntff_profile_hook(
            _ntff_profile_via_ctypes("/opt/axon/libaxon_pjrt.so")
        )
    import concourse.bass_utils as bu

    bu.upload_artifacts = lambda tmpdir: f"local://{tmpdir}"


def kernel(z, pt):
    global LAST_EXEC_NS
    from concourse.bass_utils import run_bass_kernel_spmd

    z = np.asarray(z, dtype=np.float32)
    pt = np.asarray(pt, dtype=np.float32)

    if "nc" not in _CTX:
        _CTX["nc"] = _build_program()
    nc = _CTX["nc"]

    in_maps = _make_in_maps(z, pt)

    trace = bool(int(os.environ.get("BASS_KERNEL_TRACE", "0")))
    if trace:
        try:
            _install_ntff_shim()
        except Exception:
            trace = False
    tmpdir = os.environ.get("BASS_KERNEL_TRACE_DIR") or None
    res = run_bass_kernel_spmd(
        nc, in_maps, list(range(N_CORES)), trace=trace, tmpdir=tmpdir
    )
    LAST_EXEC_NS = res.exec_time_ns

    out = np.empty((B, L, D), np.float32)
    for core in range(N_CORES):
        b, dblk = divmod(core, ND)
        out[b, :, dblk * DBLK : (dblk + 1) * DBLK] = _unpack_out(
            res.results[core]["out_s"]
        )
    return out


# revision 34
# speedup vs baseline: 1.1009x; 1.1009x over previous
"""Trainium2 Bass kernel for DeChunking EMA (lower-triangular decay matmul).

Math: out[b,i,:] = sum_{j<=i} exp(S_i - S_j) * p_j * z[b,j,:],
with S = cumsum(log(clip(1-p))). Computed chunked-scan style (Mamba-SSD):

  - L split into C=32 chunks of Q=128.
  - Intra-chunk: out_intra = W_c^T.T @ z_c with
      W_c^T[j,i] = exp(S'_i - S'_j + log p_j) (masked to i>=j),
    where S' is S re-centered per chunk (only within-chunk differences
    matter, and small magnitudes survive the PE's fp32r mantissa split).
    The delta matrix is produced on PE by a block-diagonal stacked fp32r
    matmul: delta = 1*S'_i + (-S'_j)*1 + logp_j*1, K=3 per chunk -> K=12
    block-diagonal over a group of 4 chunks ([128,512] PSUM, 1 cycle/row).
  - Inter-chunk: chunk states H_c = U_c^T @ z_c accumulate into one
    [32,192] PSUM tile via a block-diagonal U (zero except column c of
    each [128,32] slab), then one [32,32] decay matmul forms all carry-in
    rows (carry = M2^T @ H), applied per chunk as a rank-1 PSUM-accumulated
    matmul out += A_c (x) carry_c.

All exp inputs are <= 0 by construction, so nothing overflows. The decay
weights / z / state operands run in bf16 on the PE (fp32 PSUM accumulate);
the delta stack runs in fp32r (S' re-centered + pre-rounded to bf16 hi+lo).

DRAM layouts are position-major ([Q, C*DBLK]) so every DMA moves >=3 KiB
contiguous per partition.

Sharding (8 cores, no collectives): core = (batch b in {0,1}) x (one of 4
D-blocks of 192). Each core reads z[b, :, blk] and pt[b] only.
"""

import os
import numpy as np
import ml_dtypes

B, L, D = 2, 4096, 768
Q = 128
C = L // Q           # 32 chunks
ND = 4               # D blocks per batch
DBLK = D // ND       # 192
GRP = 4              # chunks per delta/exp group
NG = C // GRP        # 8 groups
NEG = -3.0e38
N_CORES = 8
ZSLABS = [4, 8, 10, 10]  # z-load slab sizes in chunks (small first slab so
                         # the H matmuls start early and keep the PE warm)
NZDMA = len(ZSLABS)
ZPFX = [0, 4, 12, 22, 32]
NODMA = 4            # out-store DMA splits

_CTX = {}
LAST_EXEC_NS = None


def _build_program():
    import concourse.bacc as bacc
    import concourse.mybir as mybir
    from concourse import tile

    f32 = mybir.dt.float32
    f32r = mybir.dt.float32r
    bf16 = mybir.dt.bfloat16
    nc = bacc.Bacc("TRN2", target_bir_lowering=False, debug=False,
                   num_devices=N_CORES, num_swdge_queues=4)

    FD = C * DBLK  # 6144 free elems in the big position-major tiles
    z_s = nc.dram_tensor("z_s", [Q, FD], bf16, kind="ExternalInput")
    # packed aux inputs: one DMA per partition-count class
    aux12 = nc.dram_tensor("aux12", [3 * GRP, NG * Q + NG * GRP * Q], f32r,
                           kind="ExternalInput")
    aux128 = nc.dram_tensor("aux128", [Q, C], f32, kind="ExternalInput")
    auxw = nc.dram_tensor("auxw", [Q, GRP * Q + Q], bf16, kind="ExternalInput")
    aux32 = nc.dram_tensor("aux32", [C, C], f32, kind="ExternalInput")
    out_s = nc.dram_tensor("out_s", [Q, FD], bf16, kind="ExternalOutput")

    Exp = mybir.ActivationFunctionType.Exp

    with tile.TileContext(nc) as tc:
        with (
            tc.tile_pool(name="zp", bufs=1) as zp,
            tc.tile_pool(name="wp", bufs=NG) as wp,
            tc.tile_pool(name="sp", bufs=1) as sp,
            tc.tile_pool(name="dps", bufs=2, space="PSUM") as dps,
            tc.tile_pool(name="ops", bufs=3, space="PSUM") as ops,
            tc.tile_pool(name="hps", bufs=1, space="PSUM") as hps,
        ):
            # z slabs stream first on sync (they gate the H state matmuls);
            # aux12 follows (the delta matmuls need it only once H is done)
            zsl = []
            for s in range(NZDMA):
                t = zp.tile([Q, ZSLABS[s] * DBLK], bf16, tag=f"z{s}")
                nc.sync.dma_start(
                    t[:], z_s[:, ZPFX[s] * DBLK : ZPFX[s + 1] * DBLK]
                )
                zsl.append(t)
            a12 = sp.tile([3 * GRP, NG * Q + NG * GRP * Q], f32r, tag="a12")
            nc.sync.dma_start(a12[:], aux12[:])

            def zchunk(c):
                s = max(i for i in range(NZDMA) if ZPFX[i] <= c)
                r = c - ZPFX[s]
                return zsl[s][:, r * DBLK : (r + 1) * DBLK]

            # remaining aux loads ride the Activation HWDGE queue in
            # parallel, smallest (most urgent) first
            a128 = sp.tile([Q, C], f32, tag="a128")
            nc.scalar.dma_start(a128[:], aux128[:])
            a32 = sp.tile([C, C], f32, tag="a32")
            nc.scalar.dma_start(a32[:], aux32[:])
            aw = sp.tile([Q, GRP * Q + Q], bf16, tag="aw")
            nc.scalar.dma_start(aw[:], auxw[:])
            sL = a12[:, 0 : NG * Q]
            sR = a12[:, NG * Q :]
            ue = a128[:]
            d2 = a32[:]
            utri = aw[:, 0 : GRP * Q]
            idn = aw[:, GRP * Q :]

            # U block-diagonal [Q, C*C] bf16: zero it, exp the [Q, C] column
            # stack, scatter onto the diagonal (stride C+1 in the free dim)
            Ublk = sp.tile([Q, C * C], bf16, tag="Ublk")
            nc.gpsimd.memset(Ublk[:], 0.0)
            Us = sp.tile([Q, C], bf16, tag="Us")
            nc.scalar.activation(Us[:], ue, Exp)
            nc.scalar.copy(Ublk[:, 0 : C * C : C + 1], Us[:])
            M2 = sp.tile([C, C], bf16, tag="M2")
            nc.scalar.activation(M2[:], d2, Exp)

            # PE clock warmup: back-to-back dense matmuls on junk data during
            # the input-DMA window flip the HAM gate to 2.4 GHz before the
            # real work arrives (the real matmuls alone are too sparse in
            # array-duty to flip it); the real H matmuls then sustain it
            wm_sb = sp.tile([Q, 2 * DBLK], bf16, tag="wm_sb")
            nc.gpsimd.memset(wm_sb[:], 1.0)
            wm_ps = ops.tile([Q, 2 * DBLK], f32, tag="o")
            for _ in range(10):
                nc.tensor.matmul(wm_ps[:], wm_sb[:, 0:Q], wm_sb[:])

            # H state matmuls chase the z slabs as they land
            h_ps = hps.tile([C, DBLK], f32, tag="h")
            for c in range(C):
                nc.tensor.matmul(
                    h_ps[:],
                    Ublk[:, c * C : (c + 1) * C],
                    zchunk(c),
                    start=(c == 0), stop=(c == C - 1),
                    skip_group_check=True,
                )

            H = sp.tile([C, DBLK], bf16, tag="H")
            nc.vector.tensor_copy(H[:], h_ps[:])
            c_ps = hps.tile([C, DBLK], f32, tag="cps")
            nc.tensor.matmul(c_ps[:], M2[:], H[:])
            # kappa is pre-folded into M2 host-side, so c_ps already holds
            # kappa*carry; cast it and fold into row 0 of each z slab
            # (out += a (x) carry == W^T row 0 applying the rank-1 update
            # once z[0] += kappa*carry)
            cfk = sp.tile([C, DBLK], bf16, tag="cfk")
            nc.vector.tensor_copy(cfk[:], c_ps[:])
            for s in range(NZDMA):
                nc.gpsimd.dma_start(
                    zsl[s][0:1, :],
                    cfk[ZPFX[s] : ZPFX[s + 1], :],
                    accum_op=mybir.AluOpType.add,
                )

            # W^T delta matmuls (block-diagonal K=12 fp32r, N=512 -> 1
            # cycle/row) + mask + exp fill the PE while the fold completes;
            # intra-chunk output matmuls trail two delta groups behind
            wT = []
            osb = sp.tile([Q, FD], bf16, tag="osb")

            def delta_group(g):
                dp = dps.tile([Q, GRP * Q], f32, tag="dp")
                nc.tensor.matmul(
                    dp[:],
                    sL[:, g * Q : (g + 1) * Q],
                    sR[:, g * GRP * Q : (g + 1) * GRP * Q],
                    start=True, stop=False,
                )
                # tril mask applied on PE: accumulate identity @ utri_neg
                nc.tensor.matmul(dp[:], idn, utri, start=False, stop=True)
                w4 = wp.tile([Q, GRP * Q], bf16, tag="w4")
                nc.scalar.activation(w4[:], dp[:], Exp)
                wT.append(w4)

            def out_pair(p):
                o_ps = ops.tile([Q, 2 * DBLK], f32, tag="o")
                for h in range(2):
                    c = 2 * p + h
                    g, k = divmod(c, GRP)
                    nc.tensor.matmul(
                        o_ps[:, h * DBLK : (h + 1) * DBLK],
                        wT[g][:, k * Q : (k + 1) * Q],
                        zchunk(c),
                    )
                osl = slice(2 * p * DBLK, (2 * p + 2) * DBLK)
                if p % 5 == 4:
                    nc.scalar.copy(osb[:, osl], o_ps[:])
                else:
                    nc.vector.tensor_copy(osb[:, osl], o_ps[:])

            delta_group(0)
            delta_group(1)
            for g in range(2, NG):
                out_pair(2 * (g - 2))
                out_pair(2 * (g - 2) + 1)
                delta_group(g)
            for p in range(2 * (NG - 2), C // 2):
                out_pair(p)

            ssl = FD // NODMA
            for s in range(NODMA):
                nc.sync.dma_start(
                    out_s[:, s * ssl : (s + 1) * ssl],
                    osb[:, s * ssl : (s + 1) * ssl],
                )

    nc.compile()
    return nc


def _host_prep(pt_b):
    """Per-batch host-side prep of the small scan operands. pt_b: [L] f32."""
    pt_b = pt_b.astype(np.float64)
    decay = np.clip(1.0 - pt_b, 1e-12, None)
    S = np.cumsum(np.log(decay))
    logp = np.log(np.maximum(pt_b, 1e-38))
    Send = S[Q - 1 :: Q]
    Sendprev = np.concatenate([[0.0], Send[:-1]])

    Sm = S.reshape(C, Q)
    logpm = logp.reshape(C, Q)
    # Re-center S within each chunk (see module docstring) and pre-round
    # operands to bf16-hi+lo representable values so the fp32r matmul
    # decomposition is exact.
    Sc = Sm - Sm[:, :1]

    def r16(x):
        h = x.astype(ml_dtypes.bfloat16).astype(np.float64)
        l = (x - h).astype(ml_dtypes.bfloat16).astype(np.float64)
        return h + l

    Sc = r16(Sc)
    logpr = r16(logpm)

    stackL = np.zeros((3 * GRP, NG * Q), np.float32)
    stackR = np.zeros((3 * GRP, NG * GRP * Q), np.float32)
    for g in range(NG):
        for k in range(GRP):
            c = g * GRP + k
            lcol = slice(g * Q, (g + 1) * Q)
            stackL[3 * k + 0, lcol] = 1.0
            stackL[3 * k + 1, lcol] = -Sc[c]
            stackL[3 * k + 2, lcol] = logpr[c]
            rcol = slice(g * GRP * Q + k * Q, g * GRP * Q + (k + 1) * Q)
            stackR[3 * k + 0, rcol] = Sc[c]
            stackR[3 * k + 1, rcol] = 1.0
            stackR[3 * k + 2, rcol] = 1.0

    # U exponent column stack: Send_c - S_j + logp_j  -> [Q, C]
    uexp = (Send[:, None] - Sm + logpm).T.astype(np.float32)

    m_i = np.arange(C)[:, None]
    c_i = np.arange(C)[None, :]
    d2exp = np.where(m_i < c_i, Sendprev[None, :] - Send[:, None], NEG)
    d2exp = d2exp.astype(np.float32)

    # log kappa_c = S_{c,0} - Send_{c-1} - logp_r[c,0]: scaling such that
    # W^T row 0 (= exp(S'_i + logp_r[c,0])) times kappa*carry reproduces the
    # rank-1 carry term a_i*carry. Uses the device-rounded logp so the
    # coefficient reconstruction cancels exactly. Folded into the M2 decay
    # matrix exponents host-side (column c of d2exp).
    logkap = np.minimum(Sm[:, 0] - Sendprev - logpr[:, 0], 69.0)
    d2exp = (d2exp + logkap[None, :]).astype(np.float32)

    aux12 = np.concatenate([stackL, stackR], axis=1)
    aux128 = uexp
    aux32 = d2exp
    return aux12, aux128, aux32


_AUXW = None


def _get_auxw():
    """bf16 [Q, GRP*Q + Q]: tiled strict-upper NEG mask + identity."""
    global _AUXW
    if _AUXW is None:
        j = np.arange(Q)[:, None]
        i = np.arange(Q)[None, :]
        one = np.where(i >= j, 0.0, NEG)
        utri = np.tile(one, (1, GRP))
        arr = np.concatenate([utri, np.eye(Q)], axis=1)
        _AUXW = arr.astype(ml_dtypes.bfloat16)
    return _AUXW


def _make_in_maps(z, pt):
    preps = [_host_prep(pt[b]) for b in range(B)]
    in_maps = []
    for core in range(N_CORES):
        b, dblk = divmod(core, ND)
        aux12, aux128, aux32 = preps[b]
        z_slab = (
            z[b, :, dblk * DBLK : (dblk + 1) * DBLK]
            .reshape(C, Q, DBLK)
            .transpose(1, 0, 2)
            .reshape(Q, C * DBLK)
            .astype(ml_dtypes.bfloat16)
        )
        in_maps.append({
            "z_s": np.ascontiguousarray(z_slab),
            "aux12": aux12,
            "aux128": aux128,
            "aux32": aux32,
            "auxw": _get_auxw(),
        })
    return in_maps


def _unpack_out(res_core):
    """out_s [Q, C*DBLK] bf16 position-major -> [L, DBLK] f32."""
    return (
        res_core.astype(np.float32)
        .reshape(Q, C, DBLK)
        .transpose(1, 0, 2)
        .reshape(L, DBLK)
    )


def _install_ntff_shim():
    """Enable NTFF profiling under axon: shim the missing antenv.axon_hooks
    module and register the ctypes hook from trn_boot; skip the fileshare
    artifact upload (no bucket in this container)."""
    import sys
    import types
    import antenv

    if "antenv.axon_hooks" not in sys.modules:
        mod = types.ModuleType("antenv.axon_hooks")
        hook_box = [None]
        mod.set_axon_ntff_profile_hook = lambda h: hook_box.__setitem__(0, h)
        mod.get_axon_ntff_profile_hook = lambda: hook_box[0]
        mod._hook_box = hook_box
        sys.modules["antenv.axon_hooks"] = mod
        antenv.axon_hooks = mod
    mod = sys.modules["antenv.axon_hooks"]
    if mod.get_axon_ntff_profile_hook() is None:
        from trn_agent_boot.trn_boot import _ntff_profile_via_ctypes

        mod.set_axon_ntff_profile_hook(
            _ntff_profile_via_ctypes("/opt/axon/libaxon_pjrt.so")
        )
    import concourse.bass_utils as bu

    bu.upload_artifacts = lambda tmpdir: f"local://{tmpdir}"


def kernel(z, pt):
    global LAST_EXEC_NS
    from concourse.bass_utils import run_bass_kernel_spmd

    z = np.asarray(z, dtype=np.float32)
    pt = np.asarray(pt, dtype=np.float32)

    if "nc" not in _CTX:
        _CTX["nc"] = _build_program()
    nc = _CTX["nc"]

    in_maps = _make_in_maps(z, pt)

    trace = bool(int(os.environ.get("BASS_KERNEL_TRACE", "0")))
    if trace:
        try:
            _install_ntff_shim()
        except Exception:
            trace = False
    tmpdir = os.environ.get("BASS_KERNEL_TRACE_DIR") or None
    res = run_bass_kernel_spmd(
        nc, in_maps, list(range(N_CORES)), trace=trace, tmpdir=tmpdir
    )
    LAST_EXEC_NS = res.exec_time_ns

    out = np.empty((B, L, D), np.float32)
    for core in range(N_CORES):
        b, dblk = divmod(core, ND)
        out[b, :, dblk * DBLK : (dblk + 1) * DBLK] = _unpack_out(
            res.results[core]["out_s"]
        )
    return out
